# revision 1
# baseline (speedup 1.0000x reference)
"""CLIPAttention (B=4, S=1024, D=768, H=12, causal) on 8 TRN2 NeuronCores.

Sharding: core c -> (batch b = c//2, head-group g = c%2).  Each core computes
6 heads of attention for one batch over the full sequence, then a PARTIAL
output projection (contraction over its 384 features).  The host sums the
two partial Z's of each batch pair — no on-device collectives.

Host-side algebraic folds (exact):
  - softmax scale folded into Wq, bq
  - K bias dropped (softmax is shift-invariant along k)
  - V bias folded through the output projection into bo_eff = bo + Wo @ bv
  - output bias added on only the g==0 core of each pair

Device layout: activations feature-major (transposed) everywhere, so no
on-device transposes.  Scores are computed as scores^T[k, q] per k-tile over
the causal q-suffix only (exact causal flop count); softmax skips the max
subtraction (scores are bounded), the causal mask is applied as a cheap
multiplicative binary mask on exp(scores) at the single diagonal 128x128
block per k-tile, and the softmax denominators ride along the PV matmul as a
65th "ones" column appended to V.  Division by the denominator happens after
PV (reciprocal_approx_fast + GpSimd partition_broadcast), before the output
projection.  PV is issued kt-outer as wide N<=512 matmuls so each V tile's
weights stay stationary.  All matmul operands are bf16 (PSUM accumulates
f32); exp runs on ScalarE from f32 PSUM scores.
"""

import sys

import numpy as np

if "/opt/trn_rl_repo" not in sys.path:
    sys.path.insert(0, "/opt/trn_rl_repo")

B, S, D, H, HD = 4, 1024, 768, 12, 64
SCALE = HD**-0.5
NEG = -1e30
NS = [S - 128 * kt for kt in range(8)]  # q-suffix width per k-tile

_CACHE = {}


def _build_nc():
    import concourse.tile as tile
    from concourse import bacc, mybir

    f32 = mybir.dt.float32
    bf16 = mybir.dt.bfloat16
    PDT = bf16  # dtype of exp(P^T) and V (the PV matmul operands)
    Exp = mybir.ActivationFunctionType.Exp
    ADD = mybir.AluOpType.add
    MULT = mybir.AluOpType.mult

    nc = bacc.Bacc("TRN2", target_bir_lowering=False, debug=False, num_devices=8)
    y_d = nc.dram_tensor("y", [D, S], bf16, kind="ExternalInput")
    wq_d = nc.dram_tensor("wq", [D, 384], bf16, kind="ExternalInput")
    wk_d = nc.dram_tensor("wk", [D, 384], bf16, kind="ExternalInput")
    wv_d = nc.dram_tensor("wv", [D, 384], bf16, kind="ExternalInput")
    wo_d = nc.dram_tensor("wo", [384, D], bf16, kind="ExternalInput")
    bq_d = nc.dram_tensor("bq", [384], f32, kind="ExternalInput")
    bo_d = nc.dram_tensor("bo", [D], f32, kind="ExternalInput")
    mask_d = nc.dram_tensor("mask", [128, 128], bf16, kind="ExternalInput")
    z_d = nc.dram_tensor("z", [D, S], f32, kind="ExternalOutput")

    with tile.TileContext(nc) as tc:
        from contextlib import ExitStack

        with ExitStack() as ctx:
            const = ctx.enter_context(tc.tile_pool(name="const", bufs=1))
            persist = ctx.enter_context(tc.tile_pool(name="persist", bufs=1))
            ptp = ctx.enter_context(tc.tile_pool(name="ptp", bufs=3))
            small = ctx.enter_context(tc.tile_pool(name="small", bufs=6))
            zpool = ctx.enter_context(tc.tile_pool(name="zpool", bufs=3))
            proj_ps = ctx.enter_context(
                tc.tile_pool(name="proj_ps", bufs=2, space="PSUM")
            )
            sc_ps = ctx.enter_context(tc.tile_pool(name="sc_ps", bufs=2, space="PSUM"))
            at_ps = ctx.enter_context(tc.tile_pool(name="at_ps", bufs=2, space="PSUM"))

            # ---------------- constant loads (one DMA per tensor) ----------------
            wq_t = const.tile([128, 6, 384], bf16, tag="wq", name="wq")
            nc.sync.dma_start(out=wq_t, in_=wq_d.ap().rearrange("(c p) m -> p c m", p=128))
            y_sb = []
            for kc in range(6):
                t = const.tile([128, S], bf16, tag=f"y{kc}", name=f"y{kc}")
                eng = nc.sync if kc % 2 == 0 else nc.gpsimd
                eng.dma_start(out=t, in_=y_d.ap()[128 * kc : 128 * kc + 128, :])
                y_sb.append(t)
            wk_t = const.tile([128, 6, 384], bf16, tag="wk", name="wk")
            nc.sync.dma_start(out=wk_t, in_=wk_d.ap().rearrange("(c p) m -> p c m", p=128))
            wv_t = const.tile([128, 6, 384], bf16, tag="wv", name="wv")
            nc.gpsimd.dma_start(out=wv_t, in_=wv_d.ap().rearrange("(c p) m -> p c m", p=128))
            wo_t = const.tile([128, 3, D], bf16, tag="wo", name="wo")
            nc.gpsimd.dma_start(out=wo_t, in_=wo_d.ap().rearrange("(c p) m -> p c m", p=128))
            wq_sb = [wq_t[:, kc, :] for kc in range(6)]
            wk_sb = [wk_t[:, kc, :] for kc in range(6)]
            wv_sb = [wv_t[:, kc, :] for kc in range(6)]
            wo_sb = [wo_t[:, kc, :] for kc in range(3)]
            bq_t = const.tile([128, 3, 1], f32, tag="bq", name="bq")
            nc.gpsimd.dma_start(out=bq_t, in_=bq_d.ap().rearrange("(c p) -> p c", p=128)[:, :, None])
            bq_sb = [bq_t[:, m, :] for m in range(3)]
            bo_t = const.tile([128, 6, 1], f32, tag="bo", name="bo")
            nc.gpsimd.dma_start(out=bo_t, in_=bo_d.ap().rearrange("(c p) -> p c", p=128)[:, :, None])
            bo_sb = [bo_t[:, m, :] for m in range(6)]
            mask_sb = const.tile([128, 128], bf16, tag="mask", name="mask")
            nc.gpsimd.dma_start(out=mask_sb, in_=mask_d.ap())


            # preload the ACT exp table so the first real exp doesn't pay it
            warm = const.tile([1, 1], f32, tag="warm", name="warm")
            nc.vector.memset(warm, 0.0)
            nc.scalar.activation(out=warm, in_=warm, func=Exp)

            # ---------------- persistent activation tiles ----------------
            qT = [persist.tile([128, S], bf16, tag=f"q{m}", name=f"q{m}") for m in range(3)]
            kT = [persist.tile([128, S], bf16, tag=f"k{m}", name=f"k{m}") for m in range(3)]
            v_sb = [persist.tile([128, 390], PDT, tag=f"v{s}", name=f"v{s}") for s in range(8)]
            for s in range(8):
                vr = v_sb[s].rearrange("p (h c) -> p h c", c=65)
                nc.vector.memset(vr[:, :, 64:65], 1.0)
            attn_sb = [persist.tile([128, S], bf16, tag=f"at{m}", name=f"at{m}") for m in range(3)]

            # ---------------- QK projections (per m-tile) ----------------
            def qk_proj(m):
                for which in range(2):  # 0 = Q, 1 = K
                    w_sb = wq_sb if which == 0 else wk_sb
                    for n in range(2):
                        ps = proj_ps.tile([128, 512], f32, tag="proj", name="proj")
                        for kc in range(6):
                            nc.tensor.matmul(
                                ps,
                                lhsT=w_sb[kc][:, 128 * m : 128 * m + 128],
                                rhs=y_sb[kc][:, 512 * n : 512 * n + 512],
                                start=(kc == 0),
                                stop=(kc == 5),
                            )
                        dst = (qT if which == 0 else kT)[m][
                            :, 512 * n : 512 * n + 512
                        ]
                        if which == 0:
                            nc.vector.tensor_scalar_add(
                                out=dst, in0=ps, scalar1=bq_sb[m]
                            )
                        else:
                            nc.vector.tensor_copy(out=dst, in_=ps)

            # ---------------- V projection ----------------
            def v_proj():
                for s in range(8):
                    ps = proj_ps.tile([128, 384], f32, tag="proj", name="proj")
                    for kc in range(6):
                        nc.tensor.matmul(
                            ps,
                            lhsT=y_sb[kc][:, 128 * s : 128 * s + 128],
                            rhs=wv_sb[kc],
                            start=(kc == 0),
                            stop=(kc == 5),
                        )
                    nc.vector.tensor_copy(
                        out=v_sb[s].rearrange("p (h c) -> p h c", c=65)[:, :, 0:64],
                        in_=ps.rearrange("p (h c) -> p h c", c=64),
                    )

            # ---------------- scores + exp for a head pair ----------------
            def scores_pair(p):
                pts = {}
                for kt in range(8):
                    N = NS[kt]
                    if kt < 4:
                        for hh in range(2):
                            po = 64 * hh
                            sct = sc_ps.tile([128, N], f32, tag="sc", name="sc")
                            c0 = 0
                            while c0 < N:
                                w = min(512, N - c0)
                                nc.tensor.matmul(
                                    sct[:, c0 : c0 + w],
                                    lhsT=kT[p][po : po + 64, 128 * kt : 128 * kt + 128],
                                    rhs=qT[p][
                                        po : po + 64, 128 * kt + c0 : 128 * kt + c0 + w
                                    ],
                                    start=True,
                                    stop=True,
                                    tile_position=(64 * hh, 0),
                                )
                                c0 += w
                            pt = ptp.tile([128, N], PDT, tag=f"pt{kt}h{hh}", name=f"pt{kt}h{hh}")
                            nc.scalar.activation(out=pt, in_=sct, func=Exp)
                            nc.vector.tensor_tensor(
                                out=pt[:, 0:128],
                                in0=pt[:, 0:128],
                                in1=mask_sb,
                                op=MULT,
                            )
                            pts[(kt, hh)] = (pt, 0)
                    else:
                        sct = sc_ps.tile([128, 512 + N], f32, tag="sc", name="sc")
                        for hh in range(2):
                            po = 64 * hh
                            o = 512 * hh
                            nc.tensor.matmul(
                                sct[:, o : o + N],
                                lhsT=kT[p][po : po + 64, 128 * kt : 128 * kt + 128],
                                rhs=qT[p][po : po + 64, 128 * kt :],
                                start=True,
                                stop=True,
                                tile_position=(64 * hh, 0),
                            )
                        pt = ptp.tile([128, 512 + N], PDT, tag=f"pt{kt}", name=f"pt{kt}")
                        nc.scalar.activation(out=pt, in_=sct, func=Exp)
                        for hh in range(2):
                            o = 512 * hh
                            nc.vector.tensor_tensor(
                                out=pt[:, o : o + 128],
                                in0=pt[:, o : o + 128],
                                in1=mask_sb,
                                op=MULT,
                            )
                        pts[(kt, 0)] = (pt, 0)
                        pts[(kt, 1)] = (pt, 512)
                return pts

            # ---------------- PV + normalize for a head pair ----------------
            def pv_one(p, hh, Bb, pts):
                h = 2 * p + hh
                po = 64 * hh
                if Bb == 0:
                    rot = (2 * p + hh) % 3
                else:
                    rot = (2 * p + hh) % 2
                if rot == 0:
                    at = at_ps.tile([65, 512], f32, tag="at", name="at")
                elif rot == 1:
                    at = sc_ps.tile([65, 512], f32, tag="sc", name="at2")
                else:
                    at = proj_ps.tile([65, 512], f32, tag="proj", name="at3")
                Jmax = 4 * Bb + 3
                for kt in range(Jmax + 1):
                    J0 = max(kt, 4 * Bb)  # first region this ktile touches
                    nJ = Jmax - J0 + 1
                    pt, base = pts[(kt, hh)]
                    co = base + 128 * (J0 - kt)
                    nc.tensor.matmul(
                        at[0:65, 128 * (J0 - 4 * Bb) : 128 * (J0 - 4 * Bb) + 128 * nJ],
                        lhsT=v_sb[kt][:, 65 * h : 65 * h + 65],
                        rhs=pt[:, co : co + 128 * nJ],
                        start=(kt == 0),
                        stop=(kt == Jmax),
                        skip_group_check=True,
                    )
                # evacuate PSUM quickly: attn rows on ACT, den row on DVE;
                # then reciprocal + broadcast + normalize multiply off-PSUM
                au = small.tile([64, 512], f32, tag="au", name="au")
                nc.scalar.copy(out=au, in_=at[0:64, 0:512])
                den = small.tile([1, 512], f32, tag="den", name="den")
                nc.scalar.copy(out=den, in_=at[64:65, 0:512])
                r = small.tile([1, 512], f32, tag="r", name="r")
                nc.vector.reciprocal_approx_fast(out=r, in_=den)
                rb = small.tile([64, 512], f32, tag="rb", name="rb")
                nc.gpsimd.partition_broadcast(rb, r)
                nc.vector.tensor_tensor(
                    out=attn_sb[p][po : po + 64, 512 * Bb : 512 * Bb + 512],
                    in0=au,
                    in1=rb,
                    op=MULT,
                )

            # ---------------- out projection (partial) ----------------
            def out_proj(n):
                if n == 0:
                    # between PV waves: tight per-m bursts on the proj slots
                    for m in range(6):
                        ps = proj_ps.tile([128, 512], f32, tag="proj", name="proj")
                        for kc in range(3):
                            nc.tensor.matmul(
                                ps,
                                lhsT=wo_sb[kc][:, 128 * m : 128 * m + 128],
                                rhs=attn_sb[kc][:, 512 * n : 512 * n + 512],
                                start=(kc == 0),
                                stop=(kc == 2),
                            )
                        zt = zpool.tile([128, 512], f32, tag="z", name="z")
                        nc.vector.tensor_scalar_add(out=zt, in0=ps, scalar1=bo_sb[m])
                        nc.sync.dma_start(
                            out=z_d.ap()[
                                128 * m : 128 * m + 128, 512 * n : 512 * n + 512
                            ],
                            in_=zt,
                        )
                else:
                    # tail pass: all PV psum slots are free — six kc-split
                    # chains in flight; each kc wave gates only on that
                    # pair's norms, so just the last wave waits the last norm
                    pss = []
                    for m in range(6):
                        if m < 2:
                            ps = proj_ps.tile([128, 512], f32, tag="proj", name="zp")
                        elif m < 4:
                            ps = at_ps.tile([128, 512], f32, tag="at", name="za")
                        else:
                            ps = sc_ps.tile([128, 512], f32, tag="sc", name="zs")
                        pss.append(ps)
                    for kc in range(3):
                        for m in range(6):
                            nc.tensor.matmul(
                                pss[m],
                                lhsT=wo_sb[kc][:, 128 * m : 128 * m + 128],
                                rhs=attn_sb[kc][:, 512 * n : 512 * n + 512],
                                start=(kc == 0),
                                stop=(kc == 2),
                                skip_group_check=True,
                            )
                    for m in range(6):
                        zt = zpool.tile([128, 512], f32, tag="z", name="z")
                        nc.vector.tensor_scalar_add(out=zt, in0=pss[m], scalar1=bo_sb[m])
                        nc.sync.dma_start(
                            out=z_d.ap()[
                                128 * m : 128 * m + 128, 512 * n : 512 * n + 512
                            ],
                            in_=zt,
                        )

            # ---------------- emission order ----------------
            all_pts = {}
            qk_proj(0)
            all_pts[0] = scores_pair(0)
            qk_proj(1)
            all_pts[1] = scores_pair(1)
            qk_proj(2)
            all_pts[2] = scores_pair(2)
            v_proj()
            for p in range(3):
                for hh in range(2):
                    pv_one(p, hh, 0, all_pts[p])
            out_proj(0)
            for p in range(3):
                for hh in range(2):
                    pv_one(p, hh, 1, all_pts[p])
            out_proj(1)

    nc.compile()
    return nc


def _get_nc():
    if "nc" not in _CACHE:
        _CACHE["nc"] = _build_nc()
    return _CACHE["nc"]


def _host_prep(inputs):
    import ml_dtypes

    bf = ml_dtypes.bfloat16
    hs = np.ascontiguousarray(np.asarray(inputs["hidden_states"], np.float32))
    Wq = np.asarray(inputs["Wq"], np.float32)
    bq = np.asarray(inputs["bq"], np.float32)
    Wk = np.asarray(inputs["Wk"], np.float32)
    Wv = np.asarray(inputs["Wv"], np.float32)
    bv = np.asarray(inputs["bv"], np.float32)
    Wo = np.asarray(inputs["Wo"], np.float32)
    bo = np.asarray(inputs["bo"], np.float32)

    bo_eff = (bo + Wo @ bv).astype(np.float32)
    zeros_bo = np.zeros_like(bo_eff)
    mask = (np.arange(128)[:, None] <= np.arange(128)[None, :]).astype(bf)

    wq_g, wk_g, wv_g, wo_g, bq_g = [], [], [], [], []
    for g in range(2):
        r0 = 384 * g
        wq_g.append(np.ascontiguousarray((Wq[r0 : r0 + 384, :] * SCALE).T.astype(bf)))
        wk_g.append(np.ascontiguousarray(Wk[r0 : r0 + 384, :].T.astype(bf)))
        wv_g.append(np.ascontiguousarray(Wv[r0 : r0 + 384, :].T.astype(bf)))
        wo_g.append(np.ascontiguousarray(Wo[:, r0 : r0 + 384].T.astype(bf)))
        bq_g.append(np.ascontiguousarray(bq[r0 : r0 + 384] * SCALE))

    yb = [np.ascontiguousarray(hs[b].T.astype(bf)) for b in range(B)]

    in_maps = []
    for c in range(8):
        b, g = c // 2, c % 2
        in_maps.append(
            {
                "y": yb[b],
                "wq": wq_g[g],
                "wk": wk_g[g],
                "wv": wv_g[g],
                "wo": wo_g[g],
                "bq": bq_g[g],
                "bo": bo_eff if g == 0 else zeros_bo,
                "mask": mask,
            }
        )
    return in_maps


def kernel(**inputs):
    from concourse.bass_utils import run_bass_kernel_spmd

    nc = _get_nc()
    in_maps = _host_prep(inputs)
    res = run_bass_kernel_spmd(nc, in_maps, core_ids=list(range(8)))
    zs = [res.results[i]["z"] for i in range(8)]
    out = np.stack(
        [(zs[2 * b].astype(np.float32) + zs[2 * b + 1].astype(np.float32)).T
         for b in range(B)]
    )
    return np.ascontiguousarray(out.astype(np.float32))



# revision 2
# speedup vs baseline: 1.0330x; 1.0330x over previous
"""CLIPAttention (B=4, S=1024, D=768, H=12, causal) on 8 TRN2 NeuronCores.

Sharding: core c -> (batch b = c//2, head-group g = c%2).  Each core computes
6 heads of attention for one batch over the full sequence, then a PARTIAL
output projection (contraction over its 384 features).  The host sums the
two partial Z's of each batch pair — no on-device collectives.

Host-side algebraic folds (exact):
  - softmax scale folded into Wq, bq
  - K bias dropped (softmax is shift-invariant along k)
  - V bias folded through the output projection into bo_eff = bo + Wo @ bv
  - output bias added on only the g==0 core of each pair

v2 changes over the first working version:
  - input DMAs priority-ordered on the single HWDGE (sync) queue so the
    first QK-projection chains start ~4us in instead of ~15us: wq/wk are
    pre-packed per m-chunk on the host and loaded interleaved with y.
  - softmax normalize reads the PV PSUM tile directly (reciprocal of the
    denominator row in PSUM, broadcast, multiply from PSUM) — the two
    ScalarE evacuation copies are gone, halving ACT load.
  - z is written bf16 (host sums pairs in f32), halving the output DMA.
  - PV Bb=1 chains for pair 0 are emitted between the Bb=0 wave and
    out_proj(0) so the PE has work while the last normalizes drain.
"""

import sys

import numpy as np

if "/opt/trn_rl_repo" not in sys.path:
    sys.path.insert(0, "/opt/trn_rl_repo")

B, S, D, H, HD = 4, 1024, 768, 12, 64
SCALE = HD**-0.5
NS = [S - 128 * kt for kt in range(8)]  # q-suffix width per k-tile

_CACHE = {}


def _build_nc():
    import concourse.tile as tile
    from concourse import bacc, mybir

    f32 = mybir.dt.float32
    bf16 = mybir.dt.bfloat16
    PDT = bf16  # dtype of exp(P^T) and V (the PV matmul operands)
    Exp = mybir.ActivationFunctionType.Exp
    MULT = mybir.AluOpType.mult

    nc = bacc.Bacc("TRN2", target_bir_lowering=False, debug=False, num_devices=8)
    y_d = nc.dram_tensor("y", [D, S], bf16, kind="ExternalInput")
    wq_d = nc.dram_tensor("wq", [3, 128, 768], bf16, kind="ExternalInput")
    wk_d = nc.dram_tensor("wk", [3, 128, 768], bf16, kind="ExternalInput")
    wv_d = nc.dram_tensor("wv", [128, 6, 384], bf16, kind="ExternalInput")
    wo_d = nc.dram_tensor("wo", [128, 3, 768], bf16, kind="ExternalInput")
    bq_d = nc.dram_tensor("bq", [128, 3], f32, kind="ExternalInput")
    bo_d = nc.dram_tensor("bo", [128, 6], f32, kind="ExternalInput")
    mask_d = nc.dram_tensor("mask", [128, 128], bf16, kind="ExternalInput")
    z_d = nc.dram_tensor("z", [D, S], bf16, kind="ExternalOutput")

    with tile.TileContext(nc) as tc:
        from contextlib import ExitStack

        with ExitStack() as ctx:
            const = ctx.enter_context(tc.tile_pool(name="const", bufs=1))
            persist = ctx.enter_context(tc.tile_pool(name="persist", bufs=1))
            ptp = ctx.enter_context(tc.tile_pool(name="ptp", bufs=3))
            small = ctx.enter_context(tc.tile_pool(name="small", bufs=6))
            zpool = ctx.enter_context(tc.tile_pool(name="zpool", bufs=3))
            proj_ps = ctx.enter_context(
                tc.tile_pool(name="proj_ps", bufs=2, space="PSUM")
            )
            sc_ps = ctx.enter_context(tc.tile_pool(name="sc_ps", bufs=2, space="PSUM"))
            at_ps = ctx.enter_context(tc.tile_pool(name="at_ps", bufs=2, space="PSUM"))

            # ------------- constant loads, priority-ordered -------------
            # tiny constants on the gpsimd (SWDGE) queue, out of the way
            bq_t = const.tile([128, 3], f32, tag="bq", name="bq")
            nc.gpsimd.dma_start(out=bq_t, in_=bq_d.ap())
            bo_t = const.tile([128, 6], f32, tag="bo", name="bo")
            nc.gpsimd.dma_start(out=bo_t, in_=bo_d.ap())
            mask_sb = const.tile([128, 128], bf16, tag="mask", name="mask")
            nc.gpsimd.dma_start(out=mask_sb, in_=mask_d.ap())
            bq_sb = [bq_t[:, m : m + 1] for m in range(3)]
            bo_sb = [bo_t[:, m : m + 1] for m in range(6)]

            # big tensors on the sync (HWDGE) queue: FIFO per queue, so
            # emission order here IS the arrival order
            wqm = [const.tile([128, 768], bf16, tag=f"wq{m}", name=f"wq{m}") for m in range(3)]
            wkm = [const.tile([128, 768], bf16, tag=f"wk{m}", name=f"wk{m}") for m in range(3)]
            y_sb = [const.tile([128, S], bf16, tag=f"y{kc}", name=f"y{kc}") for kc in range(6)]
            nc.sync.dma_start(out=wqm[0], in_=wq_d.ap()[0])
            nc.sync.dma_start(out=wkm[0], in_=wk_d.ap()[0])
            for kc in range(6):
                nc.sync.dma_start(out=y_sb[kc], in_=y_d.ap()[128 * kc : 128 * kc + 128, :])
            nc.sync.dma_start(out=wqm[1], in_=wq_d.ap()[1])
            nc.sync.dma_start(out=wkm[1], in_=wk_d.ap()[1])
            nc.sync.dma_start(out=wqm[2], in_=wq_d.ap()[2])
            nc.sync.dma_start(out=wkm[2], in_=wk_d.ap()[2])
            wv_t = const.tile([128, 6, 384], bf16, tag="wv", name="wv")
            nc.sync.dma_start(out=wv_t, in_=wv_d.ap())
            wo_t = const.tile([128, 3, 768], bf16, tag="wo", name="wo")
            nc.sync.dma_start(out=wo_t, in_=wo_d.ap())
            wv_sb = [wv_t[:, kc, :] for kc in range(6)]
            wo_sb = [wo_t[:, kc, :] for kc in range(3)]

            # preload the ACT exp table so the first real exp doesn't pay it
            warm = const.tile([1, 1], f32, tag="warm", name="warm")
            nc.vector.memset(warm, 0.0)
            nc.scalar.activation(out=warm, in_=warm, func=Exp)

            # ---------------- persistent activation tiles ----------------
            qT = [persist.tile([128, S], bf16, tag=f"q{m}", name=f"q{m}") for m in range(3)]
            kT = [persist.tile([128, S], bf16, tag=f"k{m}", name=f"k{m}") for m in range(3)]
            v_sb = [persist.tile([128, 390], PDT, tag=f"v{s}", name=f"v{s}") for s in range(8)]
            for s in range(8):
                vr = v_sb[s].rearrange("p (h c) -> p h c", c=65)
                nc.vector.memset(vr[:, :, 64:65], 1.0)
            attn_sb = [persist.tile([128, S], bf16, tag=f"at{m}", name=f"at{m}") for m in range(3)]

            # ---------------- QK projections (per m-tile) ----------------
            def qk_proj(m):
                for which in range(2):  # 0 = Q, 1 = K
                    w_t = wqm[m] if which == 0 else wkm[m]
                    for n in range(2):
                        ps = proj_ps.tile([128, 512], f32, tag="proj", name="proj")
                        for kc in range(6):
                            nc.tensor.matmul(
                                ps,
                                lhsT=w_t[:, 128 * kc : 128 * kc + 128],
                                rhs=y_sb[kc][:, 512 * n : 512 * n + 512],
                                start=(kc == 0),
                                stop=(kc == 5),
                            )
                        dst = (qT if which == 0 else kT)[m][
                            :, 512 * n : 512 * n + 512
                        ]
                        if which == 0:
                            # Q evac + bias on DVE
                            nc.vector.tensor_scalar_add(
                                out=dst, in0=ps, scalar1=bq_sb[m]
                            )
                        else:
                            # K evac on ACT (parallel engine, faster PSUM read)
                            nc.scalar.copy(out=dst, in_=ps)

            # ---------------- V projection ----------------
            def v_proj():
                for s in range(8):
                    ps = proj_ps.tile([128, 384], f32, tag="proj", name="proj")
                    for kc in range(6):
                        nc.tensor.matmul(
                            ps,
                            lhsT=y_sb[kc][:, 128 * s : 128 * s + 128],
                            rhs=wv_sb[kc],
                            start=(kc == 0),
                            stop=(kc == 5),
                        )
                    nc.vector.tensor_copy(
                        out=v_sb[s].rearrange("p (h c) -> p h c", c=65)[:, :, 0:64],
                        in_=ps.rearrange("p (h c) -> p h c", c=64),
                    )

            # ---------------- scores + exp for a head pair ----------------
            def scores_pair(p):
                pts = {}
                for kt in range(8):
                    N = NS[kt]
                    if kt < 4:
                        for hh in range(2):
                            po = 64 * hh
                            sct = sc_ps.tile([128, N], f32, tag="sc", name="sc")
                            c0 = 0
                            while c0 < N:
                                w = min(512, N - c0)
                                nc.tensor.matmul(
                                    sct[:, c0 : c0 + w],
                                    lhsT=kT[p][po : po + 64, 128 * kt : 128 * kt + 128],
                                    rhs=qT[p][
                                        po : po + 64, 128 * kt + c0 : 128 * kt + c0 + w
                                    ],
                                    start=True,
                                    stop=True,
                                    tile_position=(64 * hh, 0),
                                )
                                c0 += w
                            pt = ptp.tile([128, N], PDT, tag=f"pt{kt}h{hh}", name=f"pt{kt}h{hh}")
                            nc.scalar.activation(out=pt, in_=sct, func=Exp)
                            nc.vector.tensor_tensor(
                                out=pt[:, 0:128],
                                in0=pt[:, 0:128],
                                in1=mask_sb,
                                op=MULT,
                            )
                            pts[(kt, hh)] = (pt, 0)
                    else:
                        sct = sc_ps.tile([128, 512 + N], f32, tag="sc", name="sc")
                        for hh in range(2):
                            po = 64 * hh
                            o = 512 * hh
                            nc.tensor.matmul(
                                sct[:, o : o + N],
                                lhsT=kT[p][po : po + 64, 128 * kt : 128 * kt + 128],
                                rhs=qT[p][po : po + 64, 128 * kt :],
                                start=True,
                                stop=True,
                                tile_position=(64 * hh, 0),
                            )
                        pt = ptp.tile([128, 512 + N], PDT, tag=f"pt{kt}", name=f"pt{kt}")
                        nc.scalar.activation(out=pt, in_=sct, func=Exp)
                        for hh in range(2):
                            o = 512 * hh
                            nc.vector.tensor_tensor(
                                out=pt[:, o : o + 128],
                                in0=pt[:, o : o + 128],
                                in1=mask_sb,
                                op=MULT,
                            )
                        pts[(kt, 0)] = (pt, 0)
                        pts[(kt, 1)] = (pt, 512)
                return pts

            # ---------------- PV + normalize for a head pair ----------------
            def pv_one(p, hh, Bb, pts):
                h = 2 * p + hh
                po = 64 * hh
                if Bb == 0:
                    rot = (2 * p + hh) % 3
                else:
                    rot = (2 * p + hh) % 2
                if rot == 0:
                    at = at_ps.tile([65, 512], f32, tag="at", name="at")
                elif rot == 1:
                    at = sc_ps.tile([65, 512], f32, tag="sc", name="at2")
                else:
                    at = proj_ps.tile([65, 512], f32, tag="proj", name="at3")
                Jmax = 4 * Bb + 3
                for kt in range(Jmax + 1):
                    J0 = max(kt, 4 * Bb)  # first region this ktile touches
                    nJ = Jmax - J0 + 1
                    pt, base = pts[(kt, hh)]
                    co = base + 128 * (J0 - kt)
                    nc.tensor.matmul(
                        at[0:65, 128 * (J0 - 4 * Bb) : 128 * (J0 - 4 * Bb) + 128 * nJ],
                        lhsT=v_sb[kt][:, 65 * h : 65 * h + 65],
                        rhs=pt[:, co : co + 128 * nJ],
                        start=(kt == 0),
                        stop=(kt == Jmax),
                        skip_group_check=True,
                    )
                # normalize straight off PSUM: reciprocal of the denominator
                # row, partition-broadcast, then one multiply PSUM -> SBUF
                r = small.tile([1, 512], f32, tag="r", name="r")
                nc.vector.reciprocal_approx_fast(out=r, in_=at[64:65, 0:512])
                rb = small.tile([64, 512], f32, tag="rb", name="rb")
                nc.gpsimd.partition_broadcast(rb, r)
                nc.vector.tensor_tensor(
                    out=attn_sb[p][po : po + 64, 512 * Bb : 512 * Bb + 512],
                    in0=at[0:64, 0:512],
                    in1=rb,
                    op=MULT,
                )

            # ---------------- out projection (partial) ----------------
            def out_proj(n):
                if n == 0:
                    # between PV waves: tight per-m bursts on the proj slots
                    for m in range(6):
                        ps = proj_ps.tile([128, 512], f32, tag="proj", name="proj")
                        for kc in range(3):
                            nc.tensor.matmul(
                                ps,
                                lhsT=wo_sb[kc][:, 128 * m : 128 * m + 128],
                                rhs=attn_sb[kc][:, 512 * n : 512 * n + 512],
                                start=(kc == 0),
                                stop=(kc == 2),
                            )
                        zt = zpool.tile([128, 512], bf16, tag="z", name="z")
                        nc.vector.tensor_scalar_add(out=zt, in0=ps, scalar1=bo_sb[m])
                        nc.sync.dma_start(
                            out=z_d.ap()[
                                128 * m : 128 * m + 128, 512 * n : 512 * n + 512
                            ],
                            in_=zt,
                        )
                else:
                    # tail pass: six kc-split chains in flight; each kc wave
                    # gates only on that pair's norms
                    pss = []
                    for m in range(6):
                        if m < 2:
                            ps = proj_ps.tile([128, 512], f32, tag="proj", name="zp")
                        elif m < 4:
                            ps = at_ps.tile([128, 512], f32, tag="at", name="za")
                        else:
                            ps = sc_ps.tile([128, 512], f32, tag="sc", name="zs")
                        pss.append(ps)
                    for kc in range(3):
                        for m in range(6):
                            nc.tensor.matmul(
                                pss[m],
                                lhsT=wo_sb[kc][:, 128 * m : 128 * m + 128],
                                rhs=attn_sb[kc][:, 512 * n : 512 * n + 512],
                                start=(kc == 0),
                                stop=(kc == 2),
                                skip_group_check=True,
                            )
                    for m in range(6):
                        zt = zpool.tile([128, 512], bf16, tag="z", name="z")
                        nc.vector.tensor_scalar_add(out=zt, in0=pss[m], scalar1=bo_sb[m])
                        nc.sync.dma_start(
                            out=z_d.ap()[
                                128 * m : 128 * m + 128, 512 * n : 512 * n + 512
                            ],
                            in_=zt,
                        )

            # ---------------- emission order ----------------
            all_pts = {}
            qk_proj(0)
            all_pts[0] = scores_pair(0)
            qk_proj(1)
            all_pts[1] = scores_pair(1)
            qk_proj(2)
            all_pts[2] = scores_pair(2)
            v_proj()
            for p in range(3):
                for hh in range(2):
                    pv_one(p, hh, 0, all_pts[p])
            for hh in range(2):
                pv_one(0, hh, 1, all_pts[0])
            out_proj(0)
            for p in range(1, 3):
                for hh in range(2):
                    pv_one(p, hh, 1, all_pts[p])
            out_proj(1)

    nc.compile()
    return nc


def _get_nc():
    if "nc" not in _CACHE:
        _CACHE["nc"] = _build_nc()
    return _CACHE["nc"]


def _host_prep(inputs):
    import ml_dtypes

    bf = ml_dtypes.bfloat16
    hs = np.ascontiguousarray(np.asarray(inputs["hidden_states"], np.float32))
    Wq = np.asarray(inputs["Wq"], np.float32)
    bq = np.asarray(inputs["bq"], np.float32)
    Wk = np.asarray(inputs["Wk"], np.float32)
    Wv = np.asarray(inputs["Wv"], np.float32)
    bv = np.asarray(inputs["bv"], np.float32)
    Wo = np.asarray(inputs["Wo"], np.float32)
    bo = np.asarray(inputs["bo"], np.float32)

    bo_eff = (bo + Wo @ bv).astype(np.float32)
    zeros_bo = np.zeros_like(bo_eff)
    mask = (np.arange(128)[:, None] <= np.arange(128)[None, :]).astype(bf)

    wq_g, wk_g, wv_g, wo_g, bq_g, bo_g = [], [], [], [], [], []
    for g in range(2):
        r0 = 384 * g
        # [768, 384] transposed weight (in-dim major), then chunked:
        # wq_m[m] = [128 p, 6 kc, 128 cols] -> [3, 128, 768]
        wqT = (Wq[r0 : r0 + 384, :] * SCALE).T.astype(bf)  # [768, 384]
        wkT = Wk[r0 : r0 + 384, :].T.astype(bf)
        # [kc, 128p, m, 128c] -> [m, 128p, kc, 128c]
        def mchunk(wT):
            a = wT.reshape(6, 128, 3, 128).transpose(2, 1, 0, 3)
            return np.ascontiguousarray(a.reshape(3, 128, 768))
        wq_g.append(mchunk(wqT))
        wk_g.append(mchunk(wkT))
        # wv: [768, 384] -> [128, 6, 384]
        wvT = Wv[r0 : r0 + 384, :].T.astype(bf)
        wv_g.append(np.ascontiguousarray(wvT.reshape(6, 128, 384).transpose(1, 0, 2)))
        # wo: [384, 768] -> [128, 3, 768]
        woT = Wo[:, r0 : r0 + 384].T.astype(bf)
        wo_g.append(np.ascontiguousarray(woT.reshape(3, 128, 768).transpose(1, 0, 2)))
        bq_g.append(np.ascontiguousarray(
            (bq[r0 : r0 + 384] * SCALE).reshape(3, 128).T.astype(np.float32)))
    bo_r = np.ascontiguousarray(bo_eff.reshape(6, 128).T.astype(np.float32))
    bo_z = np.zeros_like(bo_r)

    yb = [np.ascontiguousarray(hs[b].T.astype(bf)) for b in range(B)]

    in_maps = []
    for c in range(8):
        b, g = c // 2, c % 2
        in_maps.append(
            {
                "y": yb[b],
                "wq": wq_g[g],
                "wk": wk_g[g],
                "wv": wv_g[g],
                "wo": wo_g[g],
                "bq": bq_g[g],
                "bo": bo_r if g == 0 else bo_z,
                "mask": mask,
            }
        )
    return in_maps


def kernel(**inputs):
    from concourse.bass_utils import run_bass_kernel_spmd

    nc = _get_nc()
    in_maps = _host_prep(inputs)
    res = run_bass_kernel_spmd(nc, in_maps, core_ids=list(range(8)))
    zs = [res.results[i]["z"] for i in range(8)]
    out = np.stack(
        [(zs[2 * b].astype(np.float32) + zs[2 * b + 1].astype(np.float32)).T
         for b in range(B)]
    )
    return np.ascontiguousarray(out.astype(np.float32))


# revision 3
# speedup vs baseline: 1.0358x; 1.0027x over previous
"""CLIPAttention (B=4, S=1024, D=768, H=12, causal) on 8 TRN2 NeuronCores.

Sharding: core c -> (batch b = c//2, head-group g = c%2).  Each core computes
6 heads of attention for one batch over the full sequence, then a PARTIAL
output projection (contraction over its 384 features).  The host sums the
two partial Z's of each batch pair — no on-device collectives.

Host-side algebraic folds (exact):
  - softmax scale folded into Wq, bq
  - K bias dropped (softmax is shift-invariant along k)
  - V bias folded through the output projection into bo_eff = bo + Wo @ bv
  - output bias added on only the g==0 core of each pair

v2 changes over the first working version:
  - input DMAs priority-ordered on the single HWDGE (sync) queue so the
    first QK-projection chains start ~4us in instead of ~15us: wq/wk are
    pre-packed per m-chunk on the host and loaded interleaved with y.
  - softmax normalize reads the PV PSUM tile directly (reciprocal of the
    denominator row in PSUM, broadcast, multiply from PSUM) — the two
    ScalarE evacuation copies are gone, halving ACT load.
  - z is written bf16 (host sums pairs in f32), halving the output DMA.
  - PV Bb=1 chains for pair 0 are emitted between the Bb=0 wave and
    out_proj(0) so the PE has work while the last normalizes drain.
"""

import sys

import numpy as np

if "/opt/trn_rl_repo" not in sys.path:
    sys.path.insert(0, "/opt/trn_rl_repo")

B, S, D, H, HD = 4, 1024, 768, 12, 64
SCALE = HD**-0.5
NS = [S - 128 * kt for kt in range(8)]  # q-suffix width per k-tile

_CACHE = {}


def _build_nc():
    import concourse.tile as tile
    from concourse import bacc, mybir

    f32 = mybir.dt.float32
    bf16 = mybir.dt.bfloat16
    PDT = bf16  # dtype of exp(P^T) and V (the PV matmul operands)
    Exp = mybir.ActivationFunctionType.Exp
    MULT = mybir.AluOpType.mult

    nc = bacc.Bacc("TRN2", target_bir_lowering=False, debug=False, num_devices=8)
    y_d = nc.dram_tensor("y", [D, S], bf16, kind="ExternalInput")
    wq_d = nc.dram_tensor("wq", [3, 128, 768], bf16, kind="ExternalInput")
    wk_d = nc.dram_tensor("wk", [3, 128, 768], bf16, kind="ExternalInput")
    wv_d = nc.dram_tensor("wv", [128, 6, 384], bf16, kind="ExternalInput")
    wo_d = nc.dram_tensor("wo", [128, 3, 768], bf16, kind="ExternalInput")
    bq_d = nc.dram_tensor("bq", [128, 3], f32, kind="ExternalInput")
    bo_d = nc.dram_tensor("bo", [128, 6], f32, kind="ExternalInput")
    mask_d = nc.dram_tensor("mask", [128, 128], bf16, kind="ExternalInput")
    z_d = nc.dram_tensor("z", [D, S], bf16, kind="ExternalOutput")

    with tile.TileContext(nc) as tc:
        from contextlib import ExitStack

        with ExitStack() as ctx:
            const = ctx.enter_context(tc.tile_pool(name="const", bufs=1))
            persist = ctx.enter_context(tc.tile_pool(name="persist", bufs=1))
            ptp = ctx.enter_context(tc.tile_pool(name="ptp", bufs=3))
            small = ctx.enter_context(tc.tile_pool(name="small", bufs=6))
            zpool = ctx.enter_context(tc.tile_pool(name="zpool", bufs=3))
            proj_ps = ctx.enter_context(
                tc.tile_pool(name="proj_ps", bufs=2, space="PSUM")
            )
            sc_ps = ctx.enter_context(tc.tile_pool(name="sc_ps", bufs=2, space="PSUM"))
            at_ps = ctx.enter_context(tc.tile_pool(name="at_ps", bufs=2, space="PSUM"))

            # ------------- constant loads, priority-ordered -------------
            # tiny constants on the gpsimd (SWDGE) queue, out of the way
            bq_t = const.tile([128, 3], f32, tag="bq", name="bq")
            nc.gpsimd.dma_start(out=bq_t, in_=bq_d.ap())
            bo_t = const.tile([128, 6], f32, tag="bo", name="bo")
            nc.gpsimd.dma_start(out=bo_t, in_=bo_d.ap())
            mask_sb = const.tile([128, 128], bf16, tag="mask", name="mask")
            nc.gpsimd.dma_start(out=mask_sb, in_=mask_d.ap())
            bq_sb = [bq_t[:, m : m + 1] for m in range(3)]
            bo_sb = [bo_t[:, m : m + 1] for m in range(6)]

            # big tensors on the sync (HWDGE) queue: FIFO per queue, so
            # emission order here IS the arrival order
            wqm = [const.tile([128, 768], bf16, tag=f"wq{m}", name=f"wq{m}") for m in range(3)]
            wkm = [const.tile([128, 768], bf16, tag=f"wk{m}", name=f"wk{m}") for m in range(3)]
            y_sb = [const.tile([128, S], bf16, tag=f"y{kc}", name=f"y{kc}") for kc in range(6)]
            nc.sync.dma_start(out=wqm[0], in_=wq_d.ap()[0])
            nc.sync.dma_start(out=wkm[0], in_=wk_d.ap()[0])
            for kc in range(6):
                nc.sync.dma_start(out=y_sb[kc], in_=y_d.ap()[128 * kc : 128 * kc + 128, :])
            nc.sync.dma_start(out=wqm[1], in_=wq_d.ap()[1])
            nc.sync.dma_start(out=wkm[1], in_=wk_d.ap()[1])
            nc.sync.dma_start(out=wqm[2], in_=wq_d.ap()[2])
            nc.sync.dma_start(out=wkm[2], in_=wk_d.ap()[2])
            wv_t = const.tile([128, 6, 384], bf16, tag="wv", name="wv")
            nc.sync.dma_start(out=wv_t, in_=wv_d.ap())
            wo_t = const.tile([128, 3, 768], bf16, tag="wo", name="wo")
            nc.sync.dma_start(out=wo_t, in_=wo_d.ap())
            wv_sb = [wv_t[:, kc, :] for kc in range(6)]
            wo_sb = [wo_t[:, kc, :] for kc in range(3)]

            # preload the ACT exp table so the first real exp doesn't pay it
            warm = const.tile([1, 1], f32, tag="warm", name="warm")
            nc.vector.memset(warm, 0.0)
            nc.scalar.activation(out=warm, in_=warm, func=Exp)

            # ---------------- persistent activation tiles ----------------
            qT = [persist.tile([128, S], bf16, tag=f"q{m}", name=f"q{m}") for m in range(3)]
            kT = [persist.tile([128, S], bf16, tag=f"k{m}", name=f"k{m}") for m in range(3)]
            v_sb = [persist.tile([128, 390], PDT, tag=f"v{s}", name=f"v{s}") for s in range(8)]
            for s in range(8):
                vr = v_sb[s].rearrange("p (h c) -> p h c", c=65)
                nc.vector.memset(vr[:, :, 64:65], 1.0)
            attn_sb = [persist.tile([128, S], bf16, tag=f"at{m}", name=f"at{m}") for m in range(3)]

            # ---------------- QK projections (per m-tile) ----------------
            def qk_proj(m):
                for which in range(2):  # 0 = Q, 1 = K
                    w_t = wqm[m] if which == 0 else wkm[m]
                    for n in range(2):
                        ps = proj_ps.tile([128, 512], f32, tag="proj", name="proj")
                        for kc in range(6):
                            nc.tensor.matmul(
                                ps,
                                lhsT=w_t[:, 128 * kc : 128 * kc + 128],
                                rhs=y_sb[kc][:, 512 * n : 512 * n + 512],
                                start=(kc == 0),
                                stop=(kc == 5),
                            )
                        dst = (qT if which == 0 else kT)[m][
                            :, 512 * n : 512 * n + 512
                        ]
                        if which == 0:
                            # Q evac + bias on DVE
                            nc.vector.tensor_scalar_add(
                                out=dst, in0=ps, scalar1=bq_sb[m]
                            )
                        else:
                            # K evac on ACT (parallel engine, faster PSUM read)
                            nc.scalar.copy(out=dst, in_=ps)

            # ---------------- V projection ----------------
            def v_proj():
                for s in range(8):
                    ps = proj_ps.tile([128, 384], f32, tag="proj", name="proj")
                    for kc in range(6):
                        nc.tensor.matmul(
                            ps,
                            lhsT=y_sb[kc][:, 128 * s : 128 * s + 128],
                            rhs=wv_sb[kc],
                            start=(kc == 0),
                            stop=(kc == 5),
                        )
                    nc.vector.tensor_copy(
                        out=v_sb[s].rearrange("p (h c) -> p h c", c=65)[:, :, 0:64],
                        in_=ps.rearrange("p (h c) -> p h c", c=64),
                    )

            # ---------------- scores + exp for a head pair ----------------
            def scores_pair(p):
                pts = {}
                for kt in range(8):
                    N = NS[kt]
                    if kt < 4:
                        for hh in range(2):
                            po = 64 * hh
                            sct = sc_ps.tile([128, N], f32, tag="sc", name="sc")
                            c0 = 0
                            while c0 < N:
                                w = min(512, N - c0)
                                nc.tensor.matmul(
                                    sct[:, c0 : c0 + w],
                                    lhsT=kT[p][po : po + 64, 128 * kt : 128 * kt + 128],
                                    rhs=qT[p][
                                        po : po + 64, 128 * kt + c0 : 128 * kt + c0 + w
                                    ],
                                    start=True,
                                    stop=True,
                                    tile_position=(64 * hh, 0),
                                )
                                c0 += w
                            pt = ptp.tile([128, N], PDT, tag=f"pt{kt}h{hh}", name=f"pt{kt}h{hh}")
                            nc.scalar.activation(out=pt, in_=sct, func=Exp)
                            nc.vector.tensor_tensor(
                                out=pt[:, 0:128],
                                in0=pt[:, 0:128],
                                in1=mask_sb,
                                op=MULT,
                            )
                            pts[(kt, hh)] = (pt, 0)
                    else:
                        sct = sc_ps.tile([128, 512 + N], f32, tag="sc", name="sc")
                        for hh in range(2):
                            po = 64 * hh
                            o = 512 * hh
                            nc.tensor.matmul(
                                sct[:, o : o + N],
                                lhsT=kT[p][po : po + 64, 128 * kt : 128 * kt + 128],
                                rhs=qT[p][po : po + 64, 128 * kt :],
                                start=True,
                                stop=True,
                                tile_position=(64 * hh, 0),
                            )
                        pt = ptp.tile([128, 512 + N], PDT, tag=f"pt{kt}", name=f"pt{kt}")
                        nc.scalar.activation(out=pt, in_=sct, func=Exp)
                        for hh in range(2):
                            o = 512 * hh
                            nc.vector.tensor_tensor(
                                out=pt[:, o : o + 128],
                                in0=pt[:, o : o + 128],
                                in1=mask_sb,
                                op=MULT,
                            )
                        pts[(kt, 0)] = (pt, 0)
                        pts[(kt, 1)] = (pt, 512)
                return pts

            # ---------------- PV + normalize for a head pair ----------------
            def pv_one(p, hh, Bb, pts):
                h = 2 * p + hh
                po = 64 * hh
                if Bb == 0:
                    rot = (2 * p + hh) % 3
                else:
                    rot = (2 * p + hh) % 2
                if rot == 0:
                    at = at_ps.tile([65, 512], f32, tag="at", name="at")
                elif rot == 1:
                    at = sc_ps.tile([65, 512], f32, tag="sc", name="at2")
                else:
                    at = proj_ps.tile([65, 512], f32, tag="proj", name="at3")
                Jmax = 4 * Bb + 3
                for kt in range(Jmax + 1):
                    J0 = max(kt, 4 * Bb)  # first region this ktile touches
                    nJ = Jmax - J0 + 1
                    pt, base = pts[(kt, hh)]
                    co = base + 128 * (J0 - kt)
                    nc.tensor.matmul(
                        at[0:65, 128 * (J0 - 4 * Bb) : 128 * (J0 - 4 * Bb) + 128 * nJ],
                        lhsT=v_sb[kt][:, 65 * h : 65 * h + 65],
                        rhs=pt[:, co : co + 128 * nJ],
                        start=(kt == 0),
                        stop=(kt == Jmax),
                        skip_group_check=True,
                    )
                # normalize: den row to SBUF (ACT; reciprocal_approx_fast
                # silently misreads partition-offset / PSUM sources), then
                # recip + broadcast + one multiply straight off PSUM
                den = small.tile([1, 512], f32, tag="den", name="den")
                nc.scalar.copy(out=den, in_=at[64:65, 0:512])
                r = small.tile([1, 512], f32, tag="r", name="r")
                nc.vector.reciprocal_approx_fast(out=r, in_=den)
                rb = small.tile([64, 512], f32, tag="rb", name="rb")
                nc.gpsimd.partition_broadcast(rb, r)
                nc.vector.tensor_tensor(
                    out=attn_sb[p][po : po + 64, 512 * Bb : 512 * Bb + 512],
                    in0=at[0:64, 0:512],
                    in1=rb,
                    op=MULT,
                )

            # ---------------- out projection (partial) ----------------
            def out_proj(n):
                if n == 0:
                    # between PV waves: tight per-m bursts on the proj slots
                    for m in range(6):
                        ps = proj_ps.tile([128, 512], f32, tag="proj", name="proj")
                        for kc in range(3):
                            nc.tensor.matmul(
                                ps,
                                lhsT=wo_sb[kc][:, 128 * m : 128 * m + 128],
                                rhs=attn_sb[kc][:, 512 * n : 512 * n + 512],
                                start=(kc == 0),
                                stop=(kc == 2),
                            )
                        zt = zpool.tile([128, 512], bf16, tag="z", name="z")
                        nc.vector.tensor_scalar_add(out=zt, in0=ps, scalar1=bo_sb[m])
                        nc.sync.dma_start(
                            out=z_d.ap()[
                                128 * m : 128 * m + 128, 512 * n : 512 * n + 512
                            ],
                            in_=zt,
                        )
                else:
                    # tail pass: six kc-split chains in flight; each kc wave
                    # gates only on that pair's norms
                    pss = []
                    for m in range(6):
                        if m < 2:
                            ps = proj_ps.tile([128, 512], f32, tag="proj", name="zp")
                        elif m < 4:
                            ps = at_ps.tile([128, 512], f32, tag="at", name="za")
                        else:
                            ps = sc_ps.tile([128, 512], f32, tag="sc", name="zs")
                        pss.append(ps)
                    for kc in range(3):
                        for m in range(6):
                            nc.tensor.matmul(
                                pss[m],
                                lhsT=wo_sb[kc][:, 128 * m : 128 * m + 128],
                                rhs=attn_sb[kc][:, 512 * n : 512 * n + 512],
                                start=(kc == 0),
                                stop=(kc == 2),
                                skip_group_check=True,
                            )
                    for m in range(6):
                        zt = zpool.tile([128, 512], bf16, tag="z", name="z")
                        nc.vector.tensor_scalar_add(out=zt, in0=pss[m], scalar1=bo_sb[m])
                        nc.sync.dma_start(
                            out=z_d.ap()[
                                128 * m : 128 * m + 128, 512 * n : 512 * n + 512
                            ],
                            in_=zt,
                        )

            # ---------------- emission order ----------------
            all_pts = {}
            qk_proj(0)
            all_pts[0] = scores_pair(0)
            qk_proj(1)
            all_pts[1] = scores_pair(1)
            qk_proj(2)
            all_pts[2] = scores_pair(2)
            v_proj()
            for p in range(3):
                for hh in range(2):
                    pv_one(p, hh, 0, all_pts[p])
            for hh in range(2):
                pv_one(0, hh, 1, all_pts[0])
            out_proj(0)
            for p in range(1, 3):
                for hh in range(2):
                    pv_one(p, hh, 1, all_pts[p])
            out_proj(1)

    nc.compile()
    return nc


def _get_nc():
    if "nc" not in _CACHE:
        _CACHE["nc"] = _build_nc()
    return _CACHE["nc"]


def _host_prep(inputs):
    import ml_dtypes

    bf = ml_dtypes.bfloat16
    hs = np.ascontiguousarray(np.asarray(inputs["hidden_states"], np.float32))
    Wq = np.asarray(inputs["Wq"], np.float32)
    bq = np.asarray(inputs["bq"], np.float32)
    Wk = np.asarray(inputs["Wk"], np.float32)
    Wv = np.asarray(inputs["Wv"], np.float32)
    bv = np.asarray(inputs["bv"], np.float32)
    Wo = np.asarray(inputs["Wo"], np.float32)
    bo = np.asarray(inputs["bo"], np.float32)

    bo_eff = (bo + Wo @ bv).astype(np.float32)
    zeros_bo = np.zeros_like(bo_eff)
    mask = (np.arange(128)[:, None] <= np.arange(128)[None, :]).astype(bf)

    wq_g, wk_g, wv_g, wo_g, bq_g, bo_g = [], [], [], [], [], []
    for g in range(2):
        r0 = 384 * g
        # [768, 384] transposed weight (in-dim major), then chunked:
        # wq_m[m] = [128 p, 6 kc, 128 cols] -> [3, 128, 768]
        wqT = (Wq[r0 : r0 + 384, :] * SCALE).T.astype(bf)  # [768, 384]
        wkT = Wk[r0 : r0 + 384, :].T.astype(bf)
        # [kc, 128p, m, 128c] -> [m, 128p, kc, 128c]
        def mchunk(wT):
            a = wT.reshape(6, 128, 3, 128).transpose(2, 1, 0, 3)
            return np.ascontiguousarray(a.reshape(3, 128, 768))
        wq_g.append(mchunk(wqT))
        wk_g.append(mchunk(wkT))
        # wv: [768, 384] -> [128, 6, 384]
        wvT = Wv[r0 : r0 + 384, :].T.astype(bf)
        wv_g.append(np.ascontiguousarray(wvT.reshape(6, 128, 384).transpose(1, 0, 2)))
        # wo: [384, 768] -> [128, 3, 768]
        woT = Wo[:, r0 : r0 + 384].T.astype(bf)
        wo_g.append(np.ascontiguousarray(woT.reshape(3, 128, 768).transpose(1, 0, 2)))
        bq_g.append(np.ascontiguousarray(
            (bq[r0 : r0 + 384] * SCALE).reshape(3, 128).T.astype(np.float32)))
    bo_r = np.ascontiguousarray(bo_eff.reshape(6, 128).T.astype(np.float32))
    bo_z = np.zeros_like(bo_r)

    yb = [np.ascontiguousarray(hs[b].T.astype(bf)) for b in range(B)]

    in_maps = []
    for c in range(8):
        b, g = c // 2, c % 2
        in_maps.append(
            {
                "y": yb[b],
                "wq": wq_g[g],
                "wk": wk_g[g],
                "wv": wv_g[g],
                "wo": wo_g[g],
                "bq": bq_g[g],
                "bo": bo_r if g == 0 else bo_z,
                "mask": mask,
            }
        )
    return in_maps


def kernel(**inputs):
    from concourse.bass_utils import run_bass_kernel_spmd

    nc = _get_nc()
    in_maps = _host_prep(inputs)
    res = run_bass_kernel_spmd(nc, in_maps, core_ids=list(range(8)))
    zs = [res.results[i]["z"] for i in range(8)]
    out = np.stack(
        [(zs[2 * b].astype(np.float32) + zs[2 * b + 1].astype(np.float32)).T
         for b in range(B)]
    )
    return np.ascontiguousarray(out.astype(np.float32))


# revision 4
# speedup vs baseline: 1.0644x; 1.0277x over previous
"""CLIPAttention (B=4, S=1024, D=768, H=12, causal) on 8 TRN2 NeuronCores.

Sharding: core c -> (batch b = c//2, head-group g = c%2).  Each core computes
6 heads of attention for one batch over the full sequence, then a PARTIAL
output projection (contraction over its 384 features).  The host sums the
two partial Z's of each batch pair and adds the output bias — no on-device
collectives.

Host-side algebraic folds (exact):
  - softmax scale folded into Wq, bq
  - K bias dropped (softmax is shift-invariant along k)
  - V bias folded through the output projection into bo_eff = bo + Wo @ bv
  - bo_eff added on the HOST during the pair-sum (device z is bias-free)

v3 schedule:
  - input DMAs split across the two HWDGE queues (sync + scalar), each in
    priority order (first-needed first), so the first QK chains start as
    soon as wq0/wk0/y stream in.
  - PV Bb=0 PSUM rotation avoids the sc pool (still held by pair-2 score
    tiles waiting on exp) until the end: at, proj, at, proj, sc, sc.
  - PV Bb=1 chains interleaved between out_proj(0) m-bursts.
  - z evacuation is a pure dtype-cast copy, alternating DVE/ACT, with the
    store DMAs alternating between the two HWDGE queues.
"""

import sys

import numpy as np

if "/opt/trn_rl_repo" not in sys.path:
    sys.path.insert(0, "/opt/trn_rl_repo")

B, S, D, H, HD = 4, 1024, 768, 12, 64
SCALE = HD**-0.5
NS = [S - 128 * kt for kt in range(8)]  # q-suffix width per k-tile

_CACHE = {}


def _build_nc():
    import concourse.tile as tile
    from concourse import bacc, mybir

    f32 = mybir.dt.float32
    bf16 = mybir.dt.bfloat16
    PDT = bf16  # dtype of exp(P^T) and V (the PV matmul operands)
    Exp = mybir.ActivationFunctionType.Exp
    MULT = mybir.AluOpType.mult

    nc = bacc.Bacc("TRN2", target_bir_lowering=False, debug=False, num_devices=8)
    y_d = nc.dram_tensor("y", [D, S], bf16, kind="ExternalInput")
    wq_d = nc.dram_tensor("wq", [3, 128, 768], bf16, kind="ExternalInput")
    wk_d = nc.dram_tensor("wk", [3, 128, 768], bf16, kind="ExternalInput")
    wv_d = nc.dram_tensor("wv", [128, 6, 384], bf16, kind="ExternalInput")
    wo_d = nc.dram_tensor("wo", [128, 3, 768], bf16, kind="ExternalInput")
    bq_d = nc.dram_tensor("bq", [128, 3], f32, kind="ExternalInput")
    mask_d = nc.dram_tensor("mask", [128, 128], bf16, kind="ExternalInput")
    z_d = nc.dram_tensor("z", [D, S], bf16, kind="ExternalOutput")

    with tile.TileContext(nc) as tc:
        from contextlib import ExitStack

        with ExitStack() as ctx:
            const = ctx.enter_context(tc.tile_pool(name="const", bufs=1))
            persist = ctx.enter_context(tc.tile_pool(name="persist", bufs=1))
            ptp = ctx.enter_context(tc.tile_pool(name="ptp", bufs=3))
            small = ctx.enter_context(tc.tile_pool(name="small", bufs=6))
            zpool = ctx.enter_context(tc.tile_pool(name="zpool", bufs=4))
            proj_ps = ctx.enter_context(
                tc.tile_pool(name="proj_ps", bufs=2, space="PSUM")
            )
            sc_ps = ctx.enter_context(tc.tile_pool(name="sc_ps", bufs=2, space="PSUM"))
            at_ps = ctx.enter_context(tc.tile_pool(name="at_ps", bufs=2, space="PSUM"))

            # ------------- constant loads, priority-ordered -------------
            # tiny constants on the gpsimd (SWDGE) queue, out of the way
            bq_t = const.tile([128, 3], f32, tag="bq", name="bq")
            nc.gpsimd.dma_start(out=bq_t, in_=bq_d.ap())
            mask_sb = const.tile([128, 128], bf16, tag="mask", name="mask")
            nc.gpsimd.dma_start(out=mask_sb, in_=mask_d.ap())
            bq_sb = [bq_t[:, m : m + 1] for m in range(3)]

            # big tensors split across the two HWDGE queues (sync, scalar),
            # FIFO per queue: first-needed data at the head of each
            wqm = [const.tile([128, 768], bf16, tag=f"wq{m}", name=f"wq{m}") for m in range(3)]
            wkm = [const.tile([128, 768], bf16, tag=f"wk{m}", name=f"wk{m}") for m in range(3)]
            y_sb = [const.tile([128, S], bf16, tag=f"y{kc}", name=f"y{kc}") for kc in range(6)]
            wv_t = const.tile([128, 6, 384], bf16, tag="wv", name="wv")
            wo_t = const.tile([128, 3, 768], bf16, tag="wo", name="wo")
            nc.sync.dma_start(out=wqm[0], in_=wq_d.ap()[0])
            nc.scalar.dma_start(out=wkm[0], in_=wk_d.ap()[0])
            for kc in range(6):
                eng = nc.sync if kc % 2 == 0 else nc.scalar
                eng.dma_start(out=y_sb[kc], in_=y_d.ap()[128 * kc : 128 * kc + 128, :])
            nc.sync.dma_start(out=wqm[1], in_=wq_d.ap()[1])
            nc.scalar.dma_start(out=wkm[1], in_=wk_d.ap()[1])
            nc.sync.dma_start(out=wqm[2], in_=wq_d.ap()[2])
            nc.scalar.dma_start(out=wkm[2], in_=wk_d.ap()[2])
            nc.sync.dma_start(out=wv_t, in_=wv_d.ap())
            nc.scalar.dma_start(out=wo_t, in_=wo_d.ap())
            wv_sb = [wv_t[:, kc, :] for kc in range(6)]
            wo_sb = [wo_t[:, kc, :] for kc in range(3)]

            # preload the ACT exp table so the first real exp doesn't pay it
            warm = const.tile([1, 1], f32, tag="warm", name="warm")
            nc.vector.memset(warm, 0.0)
            nc.scalar.activation(out=warm, in_=warm, func=Exp)

            # ---------------- persistent activation tiles ----------------
            qT = [persist.tile([128, S], bf16, tag=f"q{m}", name=f"q{m}") for m in range(3)]
            kT = [persist.tile([128, S], bf16, tag=f"k{m}", name=f"k{m}") for m in range(3)]
            v_sb = [persist.tile([128, 390], PDT, tag=f"v{s}", name=f"v{s}") for s in range(8)]
            for s in range(8):
                vr = v_sb[s].rearrange("p (h c) -> p h c", c=65)
                nc.vector.memset(vr[:, :, 64:65], 1.0)
            attn_sb = [persist.tile([128, S], bf16, tag=f"at{m}", name=f"at{m}") for m in range(3)]

            # ---------------- QK projections (per m-tile) ----------------
            def qk_proj(m):
                for which in range(2):  # 0 = Q, 1 = K
                    w_t = wqm[m] if which == 0 else wkm[m]
                    for n in range(2):
                        ps = proj_ps.tile([128, 512], f32, tag="proj", name="proj")
                        for kc in range(6):
                            nc.tensor.matmul(
                                ps,
                                lhsT=w_t[:, 128 * kc : 128 * kc + 128],
                                rhs=y_sb[kc][:, 512 * n : 512 * n + 512],
                                start=(kc == 0),
                                stop=(kc == 5),
                            )
                        dst = (qT if which == 0 else kT)[m][
                            :, 512 * n : 512 * n + 512
                        ]
                        if which == 0:
                            # Q evac + bias on DVE
                            nc.vector.tensor_scalar_add(
                                out=dst, in0=ps, scalar1=bq_sb[m]
                            )
                        else:
                            # K evac on ACT (parallel engine, faster PSUM read)
                            nc.scalar.copy(out=dst, in_=ps)

            # ---------------- V projection ----------------
            def v_proj():
                for s in range(8):
                    ps = proj_ps.tile([128, 384], f32, tag="proj", name="proj")
                    for kc in range(6):
                        nc.tensor.matmul(
                            ps,
                            lhsT=y_sb[kc][:, 128 * s : 128 * s + 128],
                            rhs=wv_sb[kc],
                            start=(kc == 0),
                            stop=(kc == 5),
                        )
                    nc.vector.tensor_copy(
                        out=v_sb[s].rearrange("p (h c) -> p h c", c=65)[:, :, 0:64],
                        in_=ps.rearrange("p (h c) -> p h c", c=64),
                    )

            # ---------------- scores + exp for a head pair ----------------
            def scores_pair(p):
                pts = {}
                for kt in range(8):
                    N = NS[kt]
                    if kt < 4:
                        for hh in range(2):
                            po = 64 * hh
                            sct = sc_ps.tile([128, N], f32, tag="sc", name="sc")
                            c0 = 0
                            while c0 < N:
                                w = min(512, N - c0)
                                nc.tensor.matmul(
                                    sct[:, c0 : c0 + w],
                                    lhsT=kT[p][po : po + 64, 128 * kt : 128 * kt + 128],
                                    rhs=qT[p][
                                        po : po + 64, 128 * kt + c0 : 128 * kt + c0 + w
                                    ],
                                    start=True,
                                    stop=True,
                                    tile_position=(64 * hh, 0),
                                )
                                c0 += w
                            pt = ptp.tile([128, N], PDT, tag=f"pt{kt}h{hh}", name=f"pt{kt}h{hh}")
                            nc.scalar.activation(out=pt, in_=sct, func=Exp)
                            nc.vector.tensor_tensor(
                                out=pt[:, 0:128],
                                in0=pt[:, 0:128],
                                in1=mask_sb,
                                op=MULT,
                            )
                            pts[(kt, hh)] = (pt, 0)
                    else:
                        sct = sc_ps.tile([128, 512 + N], f32, tag="sc", name="sc")
                        for hh in range(2):
                            po = 64 * hh
                            o = 512 * hh
                            nc.tensor.matmul(
                                sct[:, o : o + N],
                                lhsT=kT[p][po : po + 64, 128 * kt : 128 * kt + 128],
                                rhs=qT[p][po : po + 64, 128 * kt :],
                                start=True,
                                stop=True,
                                tile_position=(64 * hh, 0),
                            )
                        pt = ptp.tile([128, 512 + N], PDT, tag=f"pt{kt}", name=f"pt{kt}")
                        nc.scalar.activation(out=pt, in_=sct, func=Exp)
                        for hh in range(2):
                            o = 512 * hh
                            nc.vector.tensor_tensor(
                                out=pt[:, o : o + 128],
                                in0=pt[:, o : o + 128],
                                in1=mask_sb,
                                op=MULT,
                            )
                        pts[(kt, 0)] = (pt, 0)
                        pts[(kt, 1)] = (pt, 512)
                return pts

            # ---------------- PV + normalize for a head pair ----------------
            # Bb=0 PSUM rotation: sc last (pair-2 score tiles hold it until
            # exp(p2) completes)
            ROT0 = {(0, 0): 0, (0, 1): 2, (1, 0): 0, (1, 1): 2, (2, 0): 1, (2, 1): 1}

            def pv_one(p, hh, Bb, pts):
                h = 2 * p + hh
                po = 64 * hh
                if Bb == 0:
                    rot = ROT0[(p, hh)]
                else:
                    rot = (2 * p + hh) % 2
                if rot == 0:
                    at = at_ps.tile([65, 512], f32, tag="at", name="at")
                elif rot == 1:
                    at = sc_ps.tile([65, 512], f32, tag="sc", name="at2")
                else:
                    at = proj_ps.tile([65, 512], f32, tag="proj", name="at3")
                Jmax = 4 * Bb + 3
                for kt in range(Jmax + 1):
                    J0 = max(kt, 4 * Bb)  # first region this ktile touches
                    nJ = Jmax - J0 + 1
                    pt, base = pts[(kt, hh)]
                    co = base + 128 * (J0 - kt)
                    nc.tensor.matmul(
                        at[0:65, 128 * (J0 - 4 * Bb) : 128 * (J0 - 4 * Bb) + 128 * nJ],
                        lhsT=v_sb[kt][:, 65 * h : 65 * h + 65],
                        rhs=pt[:, co : co + 128 * nJ],
                        start=(kt == 0),
                        stop=(kt == Jmax),
                        skip_group_check=True,
                    )
                # normalize: den row to SBUF (ACT; reciprocal_approx_fast
                # silently misreads partition-offset / PSUM sources), then
                # recip + broadcast + one multiply straight off PSUM
                den = small.tile([1, 512], f32, tag="den", name="den")
                nc.scalar.copy(out=den, in_=at[64:65, 0:512])
                r = small.tile([1, 512], f32, tag="r", name="r")
                nc.vector.reciprocal_approx_fast(out=r, in_=den)
                rb = small.tile([64, 512], f32, tag="rb", name="rb")
                nc.gpsimd.partition_broadcast(rb, r)
                nc.vector.tensor_tensor(
                    out=attn_sb[p][po : po + 64, 512 * Bb : 512 * Bb + 512],
                    in0=at[0:64, 0:512],
                    in1=rb,
                    op=MULT,
                )

            # ------------- z evacuation (pure cast) + store -------------
            def z_out(ps, m, n, zi):
                zt = zpool.tile([128, 512], bf16, tag="z", name="z")
                if zi % 2 == 0:
                    nc.vector.tensor_copy(out=zt, in_=ps)
                else:
                    nc.scalar.copy(out=zt, in_=ps)
                eng = nc.sync if zi % 2 == 0 else nc.scalar
                eng.dma_start(
                    out=z_d.ap()[128 * m : 128 * m + 128, 512 * n : 512 * n + 512],
                    in_=zt,
                )

            # ---------------- out projection (partial) ----------------
            def op0_burst(ms):
                for m in ms:
                    ps = proj_ps.tile([128, 512], f32, tag="proj", name="proj")
                    for kc in range(3):
                        nc.tensor.matmul(
                            ps,
                            lhsT=wo_sb[kc][:, 128 * m : 128 * m + 128],
                            rhs=attn_sb[kc][:, 0:512],
                            start=(kc == 0),
                            stop=(kc == 2),
                        )
                    z_out(ps, m, 0, m)

            def out_proj1():
                pss = []
                for m in range(6):
                    if m < 2:
                        ps = proj_ps.tile([128, 512], f32, tag="proj", name="zp")
                    elif m < 4:
                        ps = at_ps.tile([128, 512], f32, tag="at", name="za")
                    else:
                        ps = sc_ps.tile([128, 512], f32, tag="sc", name="zs")
                    pss.append(ps)
                for kc in range(2):
                    for m in range(6):
                        nc.tensor.matmul(
                            pss[m],
                            lhsT=wo_sb[kc][:, 128 * m : 128 * m + 128],
                            rhs=attn_sb[kc][:, 512:1024],
                            start=(kc == 0),
                            stop=False,
                            skip_group_check=True,
                        )
                # final kc wave interleaved with per-m evacuation
                for m in range(6):
                    nc.tensor.matmul(
                        pss[m],
                        lhsT=wo_sb[2][:, 128 * m : 128 * m + 128],
                        rhs=attn_sb[2][:, 512:1024],
                        start=False,
                        stop=True,
                        skip_group_check=True,
                    )
                    z_out(pss[m], m, 1, m)

            # ---------------- emission order ----------------
            all_pts = {}
            qk_proj(0)
            all_pts[0] = scores_pair(0)
            qk_proj(1)
            all_pts[1] = scores_pair(1)
            qk_proj(2)
            all_pts[2] = scores_pair(2)
            v_proj()
            for p in range(3):
                for hh in range(2):
                    pv_one(p, hh, 0, all_pts[p])
            pv_one(0, 0, 1, all_pts[0])
            pv_one(0, 1, 1, all_pts[0])
            op0_burst([0, 1])
            pv_one(1, 0, 1, all_pts[1])
            op0_burst([2, 3])
            pv_one(1, 1, 1, all_pts[1])
            op0_burst([4, 5])
            pv_one(2, 0, 1, all_pts[2])
            pv_one(2, 1, 1, all_pts[2])
            out_proj1()

    nc.compile()
    return nc


def _get_nc():
    if "nc" not in _CACHE:
        _CACHE["nc"] = _build_nc()
    return _CACHE["nc"]


def _host_prep(inputs):
    import ml_dtypes

    bf = ml_dtypes.bfloat16
    hs = np.ascontiguousarray(np.asarray(inputs["hidden_states"], np.float32))
    Wq = np.asarray(inputs["Wq"], np.float32)
    bq = np.asarray(inputs["bq"], np.float32)
    Wk = np.asarray(inputs["Wk"], np.float32)
    Wv = np.asarray(inputs["Wv"], np.float32)
    bv = np.asarray(inputs["bv"], np.float32)
    Wo = np.asarray(inputs["Wo"], np.float32)
    bo = np.asarray(inputs["bo"], np.float32)

    bo_eff = (bo + Wo @ bv).astype(np.float32)
    mask = (np.arange(128)[:, None] <= np.arange(128)[None, :]).astype(bf)

    wq_g, wk_g, wv_g, wo_g, bq_g = [], [], [], [], []
    for g in range(2):
        r0 = 384 * g
        wqT = (Wq[r0 : r0 + 384, :] * SCALE).T.astype(bf)  # [768, 384]
        wkT = Wk[r0 : r0 + 384, :].T.astype(bf)
        # [kc, 128p, m, 128c] -> [m, 128p, kc, 128c]
        def mchunk(wT):
            a = wT.reshape(6, 128, 3, 128).transpose(2, 1, 0, 3)
            return np.ascontiguousarray(a.reshape(3, 128, 768))
        wq_g.append(mchunk(wqT))
        wk_g.append(mchunk(wkT))
        wvT = Wv[r0 : r0 + 384, :].T.astype(bf)
        wv_g.append(np.ascontiguousarray(wvT.reshape(6, 128, 384).transpose(1, 0, 2)))
        woT = Wo[:, r0 : r0 + 384].T.astype(bf)
        wo_g.append(np.ascontiguousarray(woT.reshape(3, 128, 768).transpose(1, 0, 2)))
        bq_g.append(np.ascontiguousarray(
            (bq[r0 : r0 + 384] * SCALE).reshape(3, 128).T.astype(np.float32)))

    yb = [np.ascontiguousarray(hs[b].T.astype(bf)) for b in range(B)]

    in_maps = []
    for c in range(8):
        b, g = c // 2, c % 2
        in_maps.append(
            {
                "y": yb[b],
                "wq": wq_g[g],
                "wk": wk_g[g],
                "wv": wv_g[g],
                "wo": wo_g[g],
                "bq": bq_g[g],
                "mask": mask,
            }
        )
    return in_maps, bo_eff


def kernel(**inputs):
    from concourse.bass_utils import run_bass_kernel_spmd

    nc = _get_nc()
    in_maps, bo_eff = _host_prep(inputs)
    res = run_bass_kernel_spmd(nc, in_maps, core_ids=list(range(8)))
    zs = [res.results[i]["z"] for i in range(8)]
    out = np.stack(
        [(zs[2 * b].astype(np.float32) + zs[2 * b + 1].astype(np.float32)).T
         + bo_eff[None, :]
         for b in range(B)]
    )
    return np.ascontiguousarray(out.astype(np.float32))


# revision 5
# speedup vs baseline: 1.0997x; 1.0332x over previous
"""CLIPAttention (B=4, S=1024, D=768, H=12, causal) on 8 TRN2 NeuronCores.

Sharding: core c -> (batch b = c//2, head-group g = c%2).  Each core computes
6 heads of attention for one batch over the full sequence, then a PARTIAL
output projection (contraction over its 384 features).  The host sums the
two partial Z's of each batch pair and adds the output bias — no on-device
collectives.

Host-side algebraic folds (exact):
  - softmax scale folded into Wq, bq
  - K bias dropped (softmax is shift-invariant along k)
  - V bias folded through the output projection into bo_eff = bo + Wo @ bv
  - bo_eff added on the HOST during the pair-sum (device z is bias-free)

v4 schedule:
  - fine-grained priority DMA: wq0/wk0 split per-kc, y split in halves, so
    the first QK chains start as soon as ~160KB lands; PE warm-up dummies
    run during the fill so real chains hit the 2.4GHz clock.
  - ACT does exp + den copies only (K evac moved to DVE) — exp backlog no
    longer gates the next pair's scores.
  - normalize broadcast+multiply split into 256-col halves to shorten the
    PSUM-hold latency below the PV ring distance.
  - PV Bb=1 runs pair 2 first; out_proj(1) contracts kc in order [2,0,1]
    so the final waves never wait on the last normalizes.
"""

import sys

import numpy as np

if "/opt/trn_rl_repo" not in sys.path:
    sys.path.insert(0, "/opt/trn_rl_repo")

B, S, D, H, HD = 4, 1024, 768, 12, 64
SCALE = HD**-0.5
NS = [S - 128 * kt for kt in range(8)]  # q-suffix width per k-tile

_CACHE = {}


def _build_nc():
    import concourse.tile as tile
    from concourse import bacc, mybir

    f32 = mybir.dt.float32
    bf16 = mybir.dt.bfloat16
    PDT = bf16  # dtype of exp(P^T) and V (the PV matmul operands)
    Exp = mybir.ActivationFunctionType.Exp
    MULT = mybir.AluOpType.mult

    nc = bacc.Bacc("TRN2", target_bir_lowering=False, debug=False, num_devices=8)
    y_d = nc.dram_tensor("y", [D, S], bf16, kind="ExternalInput")
    wq_d = nc.dram_tensor("wq", [3, 128, 768], bf16, kind="ExternalInput")
    wk_d = nc.dram_tensor("wk", [3, 128, 768], bf16, kind="ExternalInput")
    wv_d = nc.dram_tensor("wv", [128, 6, 384], bf16, kind="ExternalInput")
    wo_d = nc.dram_tensor("wo", [128, 3, 768], bf16, kind="ExternalInput")
    bq_d = nc.dram_tensor("bq", [128, 3], f32, kind="ExternalInput")
    mask_d = nc.dram_tensor("mask", [128, 128], bf16, kind="ExternalInput")
    z_d = nc.dram_tensor("z", [D, S], bf16, kind="ExternalOutput")

    with tile.TileContext(nc) as tc:
        from contextlib import ExitStack

        with ExitStack() as ctx:
            const = ctx.enter_context(tc.tile_pool(name="const", bufs=1))
            persist = ctx.enter_context(tc.tile_pool(name="persist", bufs=1))
            ptp = ctx.enter_context(tc.tile_pool(name="ptp", bufs=3))
            small = ctx.enter_context(tc.tile_pool(name="small", bufs=6))
            zpool = ctx.enter_context(tc.tile_pool(name="zpool", bufs=4))
            proj_ps = ctx.enter_context(
                tc.tile_pool(name="proj_ps", bufs=2, space="PSUM")
            )
            sc_ps = ctx.enter_context(tc.tile_pool(name="sc_ps", bufs=2, space="PSUM"))
            at_ps = ctx.enter_context(tc.tile_pool(name="at_ps", bufs=2, space="PSUM"))

            # ------------- constant loads, priority-ordered -------------
            bq_t = const.tile([128, 3], f32, tag="bq", name="bq")
            nc.gpsimd.dma_start(out=bq_t, in_=bq_d.ap())
            mask_sb = const.tile([128, 128], bf16, tag="mask", name="mask")
            nc.gpsimd.dma_start(out=mask_sb, in_=mask_d.ap())
            bq_sb = [bq_t[:, m : m + 1] for m in range(3)]

            wqm = [const.tile([128, 768], bf16, tag=f"wq{m}", name=f"wq{m}") for m in range(3)]
            wkm = [const.tile([128, 768], bf16, tag=f"wk{m}", name=f"wk{m}") for m in range(3)]
            y_sb = [const.tile([128, S], bf16, tag=f"y{kc}", name=f"y{kc}") for kc in range(6)]
            wv_t = const.tile([128, 6, 384], bf16, tag="wv", name="wv")
            wo_t = const.tile([128, 3, 768], bf16, tag="wo", name="wo")

            # first wave, fine-grained: m0 weight chunks + y n0-halves
            for kc in range(3):
                c = 256 * kc
                nc.sync.dma_start(out=wqm[0][:, c : c + 256], in_=wq_d.ap()[0][:, c : c + 256])
                nc.scalar.dma_start(out=wkm[0][:, c : c + 256], in_=wk_d.ap()[0][:, c : c + 256])
                for half in range(2):
                    kcc = 2 * kc + half
                    eng = nc.sync if half == 0 else nc.scalar
                    eng.dma_start(
                        out=y_sb[kcc][:, 0:512],
                        in_=y_d.ap()[128 * kcc : 128 * kcc + 128, 0:512],
                    )
            # second wave: y n1-halves
            for kcc in range(6):
                eng = nc.sync if kcc % 2 == 0 else nc.scalar
                eng.dma_start(
                    out=y_sb[kcc][:, 512:1024],
                    in_=y_d.ap()[128 * kcc : 128 * kcc + 128, 512:1024],
                )
            # remaining weights
            nc.sync.dma_start(out=wqm[1], in_=wq_d.ap()[1])
            nc.scalar.dma_start(out=wkm[1], in_=wk_d.ap()[1])
            nc.sync.dma_start(out=wqm[2], in_=wq_d.ap()[2])
            nc.scalar.dma_start(out=wkm[2], in_=wk_d.ap()[2])
            nc.sync.dma_start(out=wv_t, in_=wv_d.ap())
            nc.scalar.dma_start(out=wo_t, in_=wo_d.ap())
            wv_sb = [wv_t[:, kc, :] for kc in range(6)]
            wo_sb = [wo_t[:, kc, :] for kc in range(3)]

            # preload the ACT exp table so the first real exp doesn't pay it
            warm = const.tile([1, 1], f32, tag="warm", name="warm")
            nc.vector.memset(warm, 0.0)
            nc.scalar.activation(out=warm, in_=warm, func=Exp)

            # PE warm-up: ~3.4us of dummy matmuls during the DMA fill gets
            # the HAM clock gate to 2.4GHz before the real chains arrive
            wd = const.tile([128, 128], bf16, tag="wd", name="wd")
            nc.vector.memset(wd, 0.0)
            xd = const.tile([128, 512], bf16, tag="xd", name="xd")
            nc.vector.memset(xd, 0.0)
            psd = at_ps.tile([128, 512], f32, tag="at", name="psd")
            for _ in range(8):
                nc.tensor.matmul(
                    psd, lhsT=wd, rhs=xd, start=True, stop=True,
                    skip_group_check=True,
                )

            # ---------------- persistent activation tiles ----------------
            qT = [persist.tile([128, S], bf16, tag=f"q{m}", name=f"q{m}") for m in range(3)]
            kT = [persist.tile([128, S], bf16, tag=f"k{m}", name=f"k{m}") for m in range(3)]
            v_sb = [persist.tile([128, 390], PDT, tag=f"v{s}", name=f"v{s}") for s in range(8)]
            for s in range(8):
                vr = v_sb[s].rearrange("p (h c) -> p h c", c=65)
                nc.vector.memset(vr[:, :, 64:65], 1.0)
            attn_sb = [persist.tile([128, S], bf16, tag=f"at{m}", name=f"at{m}") for m in range(3)]

            # ---------------- QK projections (per m-tile) ----------------
            def qk_proj(m):
                for which in range(2):  # 0 = Q, 1 = K
                    w_t = wqm[m] if which == 0 else wkm[m]
                    for n in range(2):
                        ps = proj_ps.tile([128, 512], f32, tag="proj", name="proj")
                        for kc in range(6):
                            nc.tensor.matmul(
                                ps,
                                lhsT=w_t[:, 128 * kc : 128 * kc + 128],
                                rhs=y_sb[kc][:, 512 * n : 512 * n + 512],
                                start=(kc == 0),
                                stop=(kc == 5),
                            )
                        dst = (qT if which == 0 else kT)[m][
                            :, 512 * n : 512 * n + 512
                        ]
                        if which == 0:
                            nc.vector.tensor_scalar_add(
                                out=dst, in0=ps, scalar1=bq_sb[m]
                            )
                        else:
                            nc.vector.tensor_copy(out=dst, in_=ps)

            # ---------------- V projection ----------------
            def v_proj():
                for s in range(8):
                    ps = proj_ps.tile([128, 384], f32, tag="proj", name="proj")
                    for kc in range(6):
                        nc.tensor.matmul(
                            ps,
                            lhsT=y_sb[kc][:, 128 * s : 128 * s + 128],
                            rhs=wv_sb[kc],
                            start=(kc == 0),
                            stop=(kc == 5),
                        )
                    nc.vector.tensor_copy(
                        out=v_sb[s].rearrange("p (h c) -> p h c", c=65)[:, :, 0:64],
                        in_=ps.rearrange("p (h c) -> p h c", c=64),
                    )

            # ---------------- scores + exp for a head pair ----------------
            def scores_pair(p):
                pts = {}
                for kt in range(8):
                    N = NS[kt]
                    if kt < 4:
                        for hh in range(2):
                            po = 64 * hh
                            sct = sc_ps.tile([128, N], f32, tag="sc", name="sc")
                            c0 = 0
                            while c0 < N:
                                w = min(512, N - c0)
                                nc.tensor.matmul(
                                    sct[:, c0 : c0 + w],
                                    lhsT=kT[p][po : po + 64, 128 * kt : 128 * kt + 128],
                                    rhs=qT[p][
                                        po : po + 64, 128 * kt + c0 : 128 * kt + c0 + w
                                    ],
                                    start=True,
                                    stop=True,
                                    tile_position=(64 * hh, 0),
                                )
                                c0 += w
                            pt = ptp.tile([128, N], PDT, tag=f"pt{kt}h{hh}", name=f"pt{kt}h{hh}")
                            nc.scalar.activation(out=pt, in_=sct, func=Exp)
                            nc.vector.tensor_tensor(
                                out=pt[:, 0:128],
                                in0=pt[:, 0:128],
                                in1=mask_sb,
                                op=MULT,
                            )
                            pts[(kt, hh)] = (pt, 0)
                    else:
                        sct = sc_ps.tile([128, 512 + N], f32, tag="sc", name="sc")
                        for hh in range(2):
                            po = 64 * hh
                            o = 512 * hh
                            nc.tensor.matmul(
                                sct[:, o : o + N],
                                lhsT=kT[p][po : po + 64, 128 * kt : 128 * kt + 128],
                                rhs=qT[p][po : po + 64, 128 * kt :],
                                start=True,
                                stop=True,
                                tile_position=(64 * hh, 0),
                            )
                        pt = ptp.tile([128, 512 + N], PDT, tag=f"pt{kt}", name=f"pt{kt}")
                        nc.scalar.activation(out=pt, in_=sct, func=Exp)
                        for hh in range(2):
                            o = 512 * hh
                            nc.vector.tensor_tensor(
                                out=pt[:, o : o + 128],
                                in0=pt[:, o : o + 128],
                                in1=mask_sb,
                                op=MULT,
                            )
                        pts[(kt, 0)] = (pt, 0)
                        pts[(kt, 1)] = (pt, 512)
                return pts

            # ---------------- PV + normalize for a head pair ----------------
            ROT0 = {(0, 0): 0, (0, 1): 2, (1, 0): 0, (1, 1): 2, (2, 0): 1, (2, 1): 1}

            def pv_one(p, hh, Bb, pts, rot=None):
                h = 2 * p + hh
                po = 64 * hh
                if rot is None:
                    rot = ROT0[(p, hh)] if Bb == 0 else (2 * p + hh) % 2
                if rot == 0:
                    at = at_ps.tile([65, 512], f32, tag="at", name="at")
                elif rot == 1:
                    at = sc_ps.tile([65, 512], f32, tag="sc", name="at2")
                else:
                    at = proj_ps.tile([65, 512], f32, tag="proj", name="at3")
                Jmax = 4 * Bb + 3
                for kt in range(Jmax + 1):
                    J0 = max(kt, 4 * Bb)  # first region this ktile touches
                    nJ = Jmax - J0 + 1
                    pt, base = pts[(kt, hh)]
                    co = base + 128 * (J0 - kt)
                    nc.tensor.matmul(
                        at[0:65, 128 * (J0 - 4 * Bb) : 128 * (J0 - 4 * Bb) + 128 * nJ],
                        lhsT=v_sb[kt][:, 65 * h : 65 * h + 65],
                        rhs=pt[:, co : co + 128 * nJ],
                        start=(kt == 0),
                        stop=(kt == Jmax),
                        skip_group_check=True,
                    )
                # normalize: den row to SBUF (ACT; reciprocal_approx_fast
                # silently misreads partition-offset / PSUM sources), then
                # recip + broadcast/multiply in two column halves so the
                # PSUM tile frees sooner
                den = small.tile([1, 512], f32, tag="den", name="den")
                nc.scalar.copy(out=den, in_=at[64:65, 0:512])
                r = small.tile([1, 512], f32, tag="r", name="r")
                nc.vector.reciprocal_approx_fast(out=r, in_=den)
                rb = small.tile([64, 512], f32, tag="rb", name="rb")
                for cb in range(2):
                    cs = 256 * cb
                    nc.gpsimd.partition_broadcast(rb[:, cs : cs + 256], r[:, cs : cs + 256])
                    nc.vector.tensor_tensor(
                        out=attn_sb[p][po : po + 64, 512 * Bb + cs : 512 * Bb + cs + 256],
                        in0=at[0:64, cs : cs + 256],
                        in1=rb[:, cs : cs + 256],
                        op=MULT,
                    )

            # ------------- z evacuation (pure cast) + store -------------
            def z_out(ps, m, n, zi):
                zt = zpool.tile([128, 512], bf16, tag="z", name="z")
                if zi % 2 == 0:
                    nc.vector.tensor_copy(out=zt, in_=ps)
                else:
                    nc.scalar.copy(out=zt, in_=ps)
                eng = nc.sync if zi % 2 == 0 else nc.scalar
                eng.dma_start(
                    out=z_d.ap()[128 * m : 128 * m + 128, 512 * n : 512 * n + 512],
                    in_=zt,
                )

            # ---------------- out projection (partial) ----------------
            def op0_burst(ms):
                for m in ms:
                    ps = proj_ps.tile([128, 512], f32, tag="proj", name="proj")
                    for kc in range(3):
                        nc.tensor.matmul(
                            ps,
                            lhsT=wo_sb[kc][:, 128 * m : 128 * m + 128],
                            rhs=attn_sb[kc][:, 0:512],
                            start=(kc == 0),
                            stop=(kc == 2),
                        )
                    z_out(ps, m, 0, m)

            def out_proj1():
                pss = []
                for m in range(6):
                    if m < 2:
                        ps = proj_ps.tile([128, 512], f32, tag="proj", name="zp")
                    elif m < 4:
                        ps = at_ps.tile([128, 512], f32, tag="at", name="za")
                    else:
                        ps = sc_ps.tile([128, 512], f32, tag="sc", name="zs")
                    pss.append(ps)
                # contract pair 2 first (its normalizes finished earliest),
                # pair 1 last (its normalizes finish just before)
                for kc in (2, 0):
                    for m in range(6):
                        nc.tensor.matmul(
                            pss[m],
                            lhsT=wo_sb[kc][:, 128 * m : 128 * m + 128],
                            rhs=attn_sb[kc][:, 512:1024],
                            start=(kc == 2),
                            stop=False,
                            skip_group_check=True,
                        )
                for m in range(6):
                    nc.tensor.matmul(
                        pss[m],
                        lhsT=wo_sb[1][:, 128 * m : 128 * m + 128],
                        rhs=attn_sb[1][:, 512:1024],
                        start=False,
                        stop=True,
                        skip_group_check=True,
                    )
                    z_out(pss[m], m, 1, m)

            # ---------------- emission order ----------------
            all_pts = {}
            qk_proj(0)
            all_pts[0] = scores_pair(0)
            qk_proj(1)
            all_pts[1] = scores_pair(1)
            qk_proj(2)
            all_pts[2] = scores_pair(2)
            v_proj()
            for p in range(3):
                for hh in range(2):
                    pv_one(p, hh, 0, all_pts[p])
            # Bb=1: pair 2 first so the tail never waits its normalize
            pv_one(2, 0, 1, all_pts[2], rot=0)
            pv_one(2, 1, 1, all_pts[2], rot=1)
            op0_burst([0, 1])
            pv_one(0, 0, 1, all_pts[0], rot=0)
            op0_burst([2, 3])
            pv_one(0, 1, 1, all_pts[0], rot=1)
            op0_burst([4, 5])
            pv_one(1, 0, 1, all_pts[1], rot=0)
            pv_one(1, 1, 1, all_pts[1], rot=1)
            out_proj1()

    nc.compile()
    return nc


def _get_nc():
    if "nc" not in _CACHE:
        _CACHE["nc"] = _build_nc()
    return _CACHE["nc"]


def _host_prep(inputs):
    import ml_dtypes

    bf = ml_dtypes.bfloat16
    hs = np.ascontiguousarray(np.asarray(inputs["hidden_states"], np.float32))
    Wq = np.asarray(inputs["Wq"], np.float32)
    bq = np.asarray(inputs["bq"], np.float32)
    Wk = np.asarray(inputs["Wk"], np.float32)
    Wv = np.asarray(inputs["Wv"], np.float32)
    bv = np.asarray(inputs["bv"], np.float32)
    Wo = np.asarray(inputs["Wo"], np.float32)
    bo = np.asarray(inputs["bo"], np.float32)

    bo_eff = (bo + Wo @ bv).astype(np.float32)
    mask = (np.arange(128)[:, None] <= np.arange(128)[None, :]).astype(bf)

    wq_g, wk_g, wv_g, wo_g, bq_g = [], [], [], [], []
    for g in range(2):
        r0 = 384 * g
        wqT = (Wq[r0 : r0 + 384, :] * SCALE).T.astype(bf)  # [768, 384]
        wkT = Wk[r0 : r0 + 384, :].T.astype(bf)
        # [kc, 128p, m, 128c] -> [m, 128p, kc, 128c]
        def mchunk(wT):
            a = wT.reshape(6, 128, 3, 128).transpose(2, 1, 0, 3)
            return np.ascontiguousarray(a.reshape(3, 128, 768))
        wq_g.append(mchunk(wqT))
        wk_g.append(mchunk(wkT))
        wvT = Wv[r0 : r0 + 384, :].T.astype(bf)
        wv_g.append(np.ascontiguousarray(wvT.reshape(6, 128, 384).transpose(1, 0, 2)))
        woT = Wo[:, r0 : r0 + 384].T.astype(bf)
        wo_g.append(np.ascontiguousarray(woT.reshape(3, 128, 768).transpose(1, 0, 2)))
        bq_g.append(np.ascontiguousarray(
            (bq[r0 : r0 + 384] * SCALE).reshape(3, 128).T.astype(np.float32)))

    yb = [np.ascontiguousarray(hs[b].T.astype(bf)) for b in range(B)]

    in_maps = []
    for c in range(8):
        b, g = c // 2, c % 2
        in_maps.append(
            {
                "y": yb[b],
                "wq": wq_g[g],
                "wk": wk_g[g],
                "wv": wv_g[g],
                "wo": wo_g[g],
                "bq": bq_g[g],
                "mask": mask,
            }
        )
    return in_maps, bo_eff


def kernel(**inputs):
    from concourse.bass_utils import run_bass_kernel_spmd

    nc = _get_nc()
    in_maps, bo_eff = _host_prep(inputs)
    res = run_bass_kernel_spmd(nc, in_maps, core_ids=list(range(8)))
    zs = [res.results[i]["z"] for i in range(8)]
    out = np.stack(
        [(zs[2 * b].astype(np.float32) + zs[2 * b + 1].astype(np.float32)).T
         + bo_eff[None, :]
         for b in range(B)]
    )
    return np.ascontiguousarray(out.astype(np.float32))
